# revision 36
# baseline (speedup 1.0000x reference)
"""HGAT layer Trainium2 Bass kernel (v5).

Math (per batch element b, per group pair):
  q,k,v = relu(x @ w + b) for each group
  4 masked attentions (00, 11, 01, 10), each NH=4 heads of H=32
  inner/inter = relu(attn @ wo + bo); out_g = concat(inner_g, inter_g) @ wf_g + bf_g

Design (HW-microbenchmarked op costs drive the structure):
  - All matmuls fp16 (1 cyc/col on PE), fp32 PSUM accumulation.  Scores are
    32-row tile_position packed (4 heads), av/den 32-col packed.
  - softmax exp split per score tile between two paths:
      B: DVE scalar_tensor_tensor Schraudolph: i16=(sc+CADD)*(M*mask),
         bitcast f16 == exp(sc/sqrt_dk)*mask (~1.1us/tile, one op)
      C: ACT exp (~0.9us) + GPSIMD mask multiply (~1.8us)
    (path A = ACT exp + DVE mask-mul also supported).  Mask values are
    stored as M=261.25 so one mask buffer serves all paths.
  - The softmax denominator is a separate ones32-lhsT matmul producing den
    already broadcast over each head's 32 rows; 1/den = exp(-ln(den)) via two
    full-width ACT ops (DVE reciprocal measures 6.6us/op on HW — avoided).
    Ln+Exp+Relu+Copy all live in one ACT table set (no table switching).
  - masks are not host-duplicated; head duplication is a stride-0 broadcast AP.
  - wf bias rides a ones row in cc; output leaves as f16 (host casts back).
  - Emission is globally software-pipelined: av/den matmuls lag TLAG tiles
    behind the score matmuls, pair tails are delayed TAIL_DELAY av-pops, so
    the in-order engine queues never head-of-line block.
"""

import sys

sys.path.insert(0, "/opt/trn_rl_repo")

import numpy as np

import concourse.bacc as bacc
import concourse.tile as tile
from concourse import mybir

B, N, NH, H = 32, 512, 4, 32
IN_DIM, OUT_DIM = 128, 128
NCORES = 8
BS = B // NCORES  # batch elements per core
SQRT_DK = float(np.sqrt(H))
F32 = mybir.dt.float32
F16 = mybir.dt.float16
I16 = mybir.dt.int16
ADD = mybir.AluOpType.add
MULT = mybir.AluOpType.mult
EXP = mybir.ActivationFunctionType.Exp
LN = mybir.ActivationFunctionType.Ln
RELU = mybir.ActivationFunctionType.Relu

# Schraudolph f16 constants (host-calibrated against the real score range
# [0, 13]): i16 = (sc + CADD) * (M_VAL * mask); bitcast f16 ~= exp(sc/sqrt(dk)).
M_VAL = float(np.float16(1024.0 * np.log2(np.e) / SQRT_DK))  # 261.25
C_SHIFT = 45.0
CADD = (15.0 * 1024.0 - C_SHIFT) / M_VAL
BIAS_A = float(-np.log(M_VAL))  # ACT path: exp(sc/sqrt_dk - ln M) = e_true/M

# pair p -> (q group, k/v group); mask m{qg}{kg}; wo{qg}{kg}
PAIRS = [(0, 0), (1, 1), (0, 1), (1, 0)]
# pair -> (out group, concat row offset): inner pairs at rows 0:32, inter at 32:64
PAIR_DEST = [(0, 0), (1, 0), (0, 32), (1, 32)]

# Path per (chunk c, half hh) tile, indexed t = 2*c + hh (8 per pair).
PATHS = ["B", "C", "B", "C", "B", "C", "B", "B"]
TAIL_DELAY = 2
TLAG = 4  # tiles of lag between score matmuls and av/den matmuls


def _emit_qkv(nc, pools, W, b, g, qt, kt, vt, xt_t):
    """QKV projection for (b, g): qt/kt/vt [128,512] f16 (v: [k, chunk*feat])."""
    xg = xt_t[:, 512 * g : 512 * (g + 1)]
    qp = pools["sc"].tile([128, N], F32, tag="sc", name="sc")
    nc.tensor.matmul(qp[:], W["wq"][g][:], xg, start=True, stop=True)
    nc.scalar.activation(qt[:], qp[:], RELU, bias=W["bq"][g][:])

    kp = pools["sc"].tile([128, N], F32, tag="sc", name="sc")
    nc.tensor.matmul(kp[:], W["wk"][g][:], xg, start=True, stop=True)
    nc.scalar.activation(kt[:], kp[:], RELU, bias=W["bk"][g][:])

    vp = pools["sc"].tile([128, N], F32, tag="sc", name="sc")
    # ones-row bias write opens the accumulation group (bias varies along free)
    nc.tensor.matmul(vp[:], W["onesrow"][:], W["bvr4"][g][:], start=True, stop=False)
    for c in range(4):
        nc.tensor.matmul(
            vp[:, 128 * c : 128 * (c + 1)],
            xt_t[:, 512 * g + 128 * c : 512 * g + 128 * (c + 1)],
            W["wv"][g][:],
            start=False,
            stop=False,
        )
    nc.tensor.matmul(vp[:], W["zrow16"][:], W["bvr4"][g][:], start=False, stop=True)
    nc.scalar.activation(vt[:], vp[:], RELU)


def _attn_helpers(nc, pools, W):
    """emit_front/emit_av/emit_tail closures; ctx = (mt_t, qt, kt, vt, cc)."""
    avt = {}

    def emit_front(ctx, t):
        mt_t, qt, kt, vt, cc = ctx
        b, p, c, hh = t
        qg, kg = PAIRS[p]
        moff = (p * 4 + c) * 512
        mask_ap = mt_t[:, moff : moff + 512][:, None, :].broadcast_to([128, 2, N])
        sc = pools["sc"].tile([128, 2, N], F32, tag="sc", name="sc")
        for j in range(2):
            h = 2 * hh + j
            nc.tensor.matmul(
                sc[:, j],
                kt[kg][32 * h : 32 * (h + 1), 128 * c : 128 * (c + 1)],
                qt[qg][32 * h : 32 * (h + 1), :],
                start=True,
                stop=True,
                tile_position=(32 * h, 0),
            )
        path = PATHS[2 * c + hh]
        if path == "B":
            pti = pools["pt"].tile([128, 2, N], I16, tag="ptB", name="ptB")
            nc.vector.scalar_tensor_tensor(
                pti[:], sc[:], CADD, mask_ap, op0=ADD, op1=MULT
            )
            pt_mm = [pti[:, j].bitcast(F16) for j in range(2)]
        else:
            e = pools["e"].tile([128, 2, N], F16, tag="e", name="e")
            nc.scalar.activation(
                e[:], sc[:], EXP, scale=1.0 / SQRT_DK, bias=W["biasA"][:]
            )
            pt = pools["pt"].tile([128, 2, N], F16, tag="ptA", name="ptA")
            eng = nc.vector if path == "A" else nc.gpsimd
            eng.tensor_tensor(pt[:], e[:], mask_ap, op=MULT)
            pt_mm = [pt[:, j] for j in range(2)]
        return (t, pt_mm)

    def emit_av(ctx, work):
        mt_t, qt, kt, vt, cc = ctx
        (b, p, c, hh), pt_mm = work
        kg = PAIRS[p][1]
        key = (b, p)
        if key not in avt:
            av = pools["av"].tile([128, N], F32, tag="av", name="av")
            den = pools["den"].tile([128, N], F32, tag="den", name="den")
            nc.tensor.matmul(av[:], W["zrow16"][:], W["bvr4"][0][:], start=True, stop=False)
            nc.tensor.matmul(den[:], W["zrow16"][:], W["bvr4"][0][:], start=True, stop=False)
            avt[key] = (av, den)
        av, den = avt[key]
        for j in range(2):
            h = 2 * hh + j
            nc.tensor.matmul(
                av[32 * h : 32 * (h + 1), :],
                vt[kg][:, 128 * c + 32 * h : 128 * c + 32 * (h + 1)],
                pt_mm[j],
                start=False,
                stop=False,
                tile_position=(0, 32 * h),
            )
            nc.tensor.matmul(
                den[32 * h : 32 * (h + 1), :],
                W["ones32"][:],
                pt_mm[j],
                start=False,
                stop=False,
                tile_position=(0, 32 * h),
            )

    def emit_tail(ctx, b, p):
        mt_t, qt, kt, vt, cc = ctx
        av, den = avt.pop((b, p))
        nc.tensor.matmul(av[:], W["zrow16"][:], W["bvr4"][0][:], start=False, stop=True)
        nc.tensor.matmul(den[:], W["zrow16"][:], W["bvr4"][0][:], start=False, stop=True)
        # 1/den = exp(-ln(den)) on ACT: den is broadcast over head rows, so two
        # full-width ACT ops cover all four heads (DVE reciprocal is 6.6us/op)
        u = pools["u"].tile([128, N], F32, tag="u", name="u")
        nc.scalar.activation(u[:], den[:], LN)
        rden = pools["rden"].tile([128, N], F16, tag="rden", name="rden")
        with nc.allow_low_precision(reason="1/den in f16 is plenty for softmax"):
            nc.scalar.activation(rden[:], u[:], EXP, scale=-1.0)
        an = pools["an"].tile([128, N], F16, tag="an", name="an")
        nc.vector.tensor_tensor(an[:], av[:], rden[:], op=MULT)
        g, row = PAIR_DEST[p]
        wop = pools["sc"].tile([32, N], F32, tag="sc", name="sc")
        nc.tensor.matmul(wop[:], W["wo"][p][:], an[:], start=True, stop=True)
        nc.scalar.activation(cc[g][row : row + 32, :], wop[:], RELU, bias=W["bo"][p][:])

    return emit_front, emit_av, emit_tail


def _emit_out(nc, pools, W, b, g, cc):
    wfp = pools["sc"].tile([128, N], F32, tag="sc", name="sc")
    nc.tensor.matmul(wfp[:], W["wf"][g][:], cc[g][:], start=True, stop=True)
    ot = pools["ot"].tile([128, N], F16, tag="ot", name="ot")
    nc.scalar.copy(ot[:], wfp[:])
    nc.sync.dma_start(out=W["yt_ap"][b * 2 + g], in_=ot[:])


def build_nc(n_iters: int = 1):
    """Build + compile the per-core Bass module (body repeated n_iters times)."""
    import contextlib

    nc = bacc.Bacc("TRN2", target_bir_lowering=False, debug=False)

    xt = nc.dram_tensor("xt", [BS, 128, 2 * N], F16, kind="ExternalInput")
    mt = nc.dram_tensor("mt", [BS, 128, 16 * 512], F16, kind="ExternalInput")
    wqk = nc.dram_tensor("wqk", [2, 2, 128, 128], F16, kind="ExternalInput")
    wv = nc.dram_tensor("wv", [2, 128, 128], F16, kind="ExternalInput")
    bqk = nc.dram_tensor("bqk", [2, 2, 128, 1], F32, kind="ExternalInput")
    bvr4 = nc.dram_tensor("bvr4", [2, 1, 512], F16, kind="ExternalInput")
    wo = nc.dram_tensor("wo", [4, 128, 32], F16, kind="ExternalInput")
    bo = nc.dram_tensor("bo", [4, 32, 1], F32, kind="ExternalInput")
    wf = nc.dram_tensor("wf", [2, 65, 128], F16, kind="ExternalInput")
    onesrow = nc.dram_tensor("onesrow", [1, 128], F16, kind="ExternalInput")
    ones32 = nc.dram_tensor("ones32", [128, 32], F16, kind="ExternalInput")
    yt = nc.dram_tensor("yt", [BS * 2, 128, N], F16, kind="ExternalOutput")

    with tile.TileContext(nc) as tc, contextlib.ExitStack() as ctx:
        pools = {
            "consts": ctx.enter_context(tc.tile_pool(name="consts", bufs=1)),
            "xt": ctx.enter_context(tc.tile_pool(name="xt", bufs=2)),
            "persist": ctx.enter_context(tc.tile_pool(name="persist", bufs=1)),
            "mt": ctx.enter_context(tc.tile_pool(name="mt", bufs=2)),
            "e": ctx.enter_context(tc.tile_pool(name="e", bufs=4)),
            "pt": ctx.enter_context(tc.tile_pool(name="pt", bufs=8)),
            "u": ctx.enter_context(tc.tile_pool(name="u", bufs=2)),
            "rden": ctx.enter_context(tc.tile_pool(name="rden", bufs=2)),
            "an": ctx.enter_context(tc.tile_pool(name="an", bufs=2)),
            "ot": ctx.enter_context(tc.tile_pool(name="ot", bufs=2)),
            "sc": ctx.enter_context(tc.tile_pool(name="sc", bufs=3, space="PSUM")),
            "av": ctx.enter_context(tc.tile_pool(name="av", bufs=1, space="PSUM")),
            "den": ctx.enter_context(tc.tile_pool(name="den", bufs=1, space="PSUM")),
        }
        cp = pools["consts"]
        W = {
            "yt_ap": yt.ap(),
            "wq": [cp.tile([128, 128], F16, tag=f"wq{g}", name=f"wq{g}") for g in range(2)],
            "wk": [cp.tile([128, 128], F16, tag=f"wk{g}", name=f"wk{g}") for g in range(2)],
            "wv": [cp.tile([128, 128], F16, tag=f"wv{g}", name=f"wv{g}") for g in range(2)],
            "bq": [cp.tile([128, 1], F32, tag=f"bq{g}", name=f"bq{g}") for g in range(2)],
            "bk": [cp.tile([128, 1], F32, tag=f"bk{g}", name=f"bk{g}") for g in range(2)],
            "bvr4": [cp.tile([1, 512], F16, tag=f"bvr4{g}", name=f"bvr4{g}") for g in range(2)],
            "zrow16": cp.tile([1, 128], F16, tag="zrow16", name="zrow16"),
            "wo": [cp.tile([128, 32], F16, tag=f"wo{p}", name=f"wo{p}") for p in range(4)],
            "bo": [cp.tile([32, 1], F32, tag=f"bo{p}", name=f"bo{p}") for p in range(4)],
            "wf": [cp.tile([65, 128], F16, tag=f"wf{g}", name=f"wf{g}") for g in range(2)],
            "onesrow": cp.tile([1, 128], F16, tag="onesrow", name="onesrow"),
            "ones32": cp.tile([128, 32], F16, tag="ones32", name="ones32"),
            "biasA": cp.tile([128, 1], F32, tag="biasA", name="biasA"),
        }
        nc.vector.memset(W["biasA"][:], BIAS_A)
        for g in range(2):
            nc.sync.dma_start(out=W["wq"][g][:], in_=wqk.ap()[g, 0])
            nc.sync.dma_start(out=W["wk"][g][:], in_=wqk.ap()[g, 1])
            nc.sync.dma_start(out=W["wv"][g][:], in_=wv.ap()[g])
            nc.sync.dma_start(out=W["bq"][g][:], in_=bqk.ap()[g, 0])
            nc.sync.dma_start(out=W["bk"][g][:], in_=bqk.ap()[g, 1])
            nc.sync.dma_start(out=W["bvr4"][g][:], in_=bvr4.ap()[g])
            nc.sync.dma_start(out=W["wf"][g][:], in_=wf.ap()[g])
        for p in range(4):
            nc.sync.dma_start(out=W["wo"][p][:], in_=wo.ap()[p])
            nc.sync.dma_start(out=W["bo"][p][:], in_=bo.ap()[p])
        nc.sync.dma_start(out=W["onesrow"][:], in_=onesrow.ap())
        nc.vector.memset(W["zrow16"][:], 0.0)
        nc.sync.dma_start(out=W["ones32"][:], in_=ones32.ap())

        pp = pools["persist"]
        emit_front, emit_av, emit_tail = _attn_helpers(nc, pools, W)
        for it in range(n_iters):
            sfx = ""  # reuse tile tags across iterations (bounded SBUF)

            def _qkv_tiles(b):
                q = [pp.tile([128, N], F16, tag=f"qt{b}{g}{sfx}", name=f"qt{b}{g}{sfx}") for g in range(2)]
                k = [pp.tile([128, N], F16, tag=f"kt{b}{g}{sfx}", name=f"kt{b}{g}{sfx}") for g in range(2)]
                v = [pp.tile([128, N], F16, tag=f"vt{b}{g}{sfx}", name=f"vt{b}{g}{sfx}") for g in range(2)]
                return q, k, v

            ctxs = {}

            def prep_b(b):
                if b >= BS or b in ctxs:
                    return
                xt_t = pools["xt"].tile([128, 2 * N], F16, tag="xt", name="xt")
                nc.sync.dma_start(out=xt_t[:], in_=xt.ap()[b])
                mt_t = pools["mt"].tile([128, 16 * 512], F16, tag="mt", name="mt")
                nc.sync.dma_start(out=mt_t[:], in_=mt.ap()[b])
                qt, kt, vt = _qkv_tiles(b)
                for g in range(2):
                    _emit_qkv(nc, pools, W, b, g, qt[g], kt[g], vt[g], xt_t)
                cc = [
                    pp.tile([65, N], F16, tag=f"cc{b}{g}{sfx}", name=f"cc{b}{g}{sfx}")
                    for g in range(2)
                ]
                for g in range(2):
                    nc.gpsimd.memset(cc[g][64:65, :], 1.0)
                ctxs[b] = (mt_t, qt, kt, vt, cc)

            prep_b(0)
            tiles = [
                (b, p, c, hh)
                for b in range(BS)
                for p in range(4)
                for c in range(4)
                for hh in range(2)
            ]
            pending = []
            done_av = {}
            tailq = []
            npops = [0]

            def flush_ready_tails(force=False):
                while tailq and (force or npops[0] >= tailq[0][2] + TAIL_DELAY):
                    wb, wp, _ = tailq.pop(0)
                    emit_tail(ctxs[wb], wb, wp)
                    if wp == 3:
                        for g in range(2):
                            _emit_out(nc, pools, W, wb, g, ctxs[wb][4])

            def pop_one():
                w = pending.pop(0)
                (wb, wp, _, _), _ = w
                emit_av(ctxs[wb], w)
                npops[0] += 1
                done_av[(wb, wp)] = done_av.get((wb, wp), 0) + 1
                if done_av[(wb, wp)] == 8:
                    tailq.append((wb, wp, npops[0]))
                flush_ready_tails()

            for t in tiles:
                b = t[0]
                if t[1] == 0 and t[2] == 0 and t[3] == 0:
                    prep_b(b + 1)  # staggered qkv for the next batch element
                pending.append(emit_front(ctxs[b], t))
                if len(pending) > TLAG:
                    pop_one()
            while pending:
                pop_one()
            flush_ready_tails(force=True)

    nc.compile()
    return nc


def prep_weights(inp):
    """Host-side packing of the (core-replicated) weight tensors."""
    f = np.asarray
    W = {}
    W["wqk"] = np.stack(
        [
            np.stack([f(inp["wq0"]), f(inp["wk0"])]),
            np.stack([f(inp["wq1"]), f(inp["wk1"])]),
        ]
    ).astype(np.float16)
    W["wv"] = np.stack([f(inp["wv0"]), f(inp["wv1"])]).astype(np.float16)
    W["bqk"] = np.stack(
        [
            np.stack([f(inp["bq0"]).reshape(128, 1), f(inp["bk0"]).reshape(128, 1)]),
            np.stack([f(inp["bq1"]).reshape(128, 1), f(inp["bk1"]).reshape(128, 1)]),
        ]
    ).astype(np.float32)
    W["bvr4"] = np.stack(
        [np.tile(f(inp["bv0"]), 4).reshape(1, 512), np.tile(f(inp["bv1"]), 4).reshape(1, 512)]
    ).astype(np.float16)
    W["wo"] = np.stack(
        [f(inp["wo00"]), f(inp["wo11"]), f(inp["wo01"]), f(inp["wo10"])]
    ).astype(np.float16)
    W["bo"] = np.stack(
        [
            f(inp["bo00"]).reshape(32, 1),
            f(inp["bo11"]).reshape(32, 1),
            f(inp["bo01"]).reshape(32, 1),
            f(inp["bo10"]).reshape(32, 1),
        ]
    ).astype(np.float32)
    wf_stack = []
    for g in range(2):
        wfg = np.concatenate(
            [f(inp[f"wf{g}"]), f(inp[f"bf{g}"]).reshape(1, 128)], axis=0
        )  # [65, 128]
        wf_stack.append(wfg)
    W["wf"] = np.stack(wf_stack).astype(np.float16)
    W["onesrow"] = np.ones((1, 128), np.float16)
    W["ones32"] = np.ones((128, 32), np.float16)
    return W


def prep_core_inputs(inp, W):
    """Build the 8 per-core in_maps (shards batch over cores)."""
    x = [np.asarray(inp["x0"], np.float32), np.asarray(inp["x1"], np.float32)]
    masks = [
        np.asarray(inp["m00"]),
        np.asarray(inp["m11"]),
        np.asarray(inp["m01"]),
        np.asarray(inp["m10"]),
    ]
    in_maps = []
    for ci in range(NCORES):
        xt = np.empty((BS, 128, 2 * N), np.float16)
        mtv = np.empty((BS, 128, 16 * 512), np.float16)
        for b in range(BS):
            gb = ci * BS + b
            for g in range(2):
                xt[b, :, 512 * g : 512 * (g + 1)] = x[g][gb].T
            for p in range(4):
                mT = masks[p][gb].T.astype(np.float16) * np.float16(M_VAL)  # [k, q]
                ch = mT.reshape(4, 128, N)  # chunk c = k rows 128c..
                mtv[b, :, (p * 4) * 512 : (p * 4 + 4) * 512] = (
                    ch.transpose(1, 0, 2).reshape(128, 4 * N)
                )
        m = {"xt": xt, "mt": mtv}
        m.update(W)
        in_maps.append(m)
    return in_maps


def postprocess(results):
    """Gather per-core yt [8,128,512] -> (out0, out1) full arrays."""
    out0 = np.empty((B, N, OUT_DIM), np.float32)
    out1 = np.empty((B, N, OUT_DIM), np.float32)
    for ci in range(NCORES):
        yt = results[ci]["yt"]
        for b in range(BS):
            gb = ci * BS + b
            out0[gb] = yt[b * 2 + 0].T
            out1[gb] = yt[b * 2 + 1].T
    return out0, out1


_NC_CACHE = {}


def get_nc(n_iters: int = 1):
    if n_iters not in _NC_CACHE:
        _NC_CACHE[n_iters] = build_nc(n_iters)
    return _NC_CACHE[n_iters]


def kernel(**inputs):
    from concourse import bass_utils

    nc = get_nc(1)
    W = prep_weights(inputs)
    in_maps = prep_core_inputs(inputs, W)
    res = bass_utils.run_bass_kernel_spmd(
        nc, in_maps, core_ids=list(range(NCORES)), trace=False
    )
    return postprocess(res.results)
